# revision 7
# baseline (speedup 1.0000x reference)
"""Trainium2 Bass kernel for DirCFConv-style GNN message passing, v4.

Computes, for inputs s:(B,N,H) f32, ef_mask:(B,N,N,H) f32, W:(H,H), b:(H,):
    m   = SiLU(LayerNorm(s @ W.T + b))          # (B,N,H)
    out[b,i,h] = sum_j ef_mask[b,i,j,h] * m[b,j,h]

Sharding: 8 cores, core c handles batch b = c // 2 and query-node half
i in [ (c%2)*256, (c%2)*256+256 ).  Each core streams its 64 MiB mask
shard from HBM (the roofline: 64 MiB at the 358 GB/s per-core cap is
187 us; v2's stream already ran at that rate from t=8.6..196.5us).

v2 layout (kept): SBUF partition p holds the JJ=4 consecutive j's
{4p..4p+3}, so each DMA descriptor covers a contiguous (jj,h) run of
2 KiB.  Every tile's DMA is issued as two 2 MiB halves on opposite
HWDGE queues (SP/ACT); the last three tiles use 1 MiB quarters.  The
multiply uses a partition-permuted m broadcast along i with a 0-stride
AP; the j-reduction is JJ accumulating PE matmuls per query node on a
bf16 product.  The in-loop trigger emission and queue parity are
LOAD-BEARING: prefetch-style reorderings (all triggers upfront, or a
deep single-queue prefetch) stall the issuing engine on the ~2-chunk
HWDGE descriptor ring and/or unbalance the queues, dropping the
steady-state stream from 357 GB/s to ~320 (measured 227us and 238us).

v4 changes vs v2 (222.8us):
 - s/W/b loads moved from the gpsimd SWDGE queue (slow software
   descriptors; m was only ready at t=32.7us, leaving a DVE backlog
   that ran 20us past the final DMA) to the heads of the two HWDGE
   queues, in front of the mask triggers.  Stage 1 now finishes before
   tile 0 lands and the DVE tracks the stream with no backlog.
 - Mid-stream block-0 output store moved from SWDGE (928ns per 512B
   descriptor, a 15us trickle) to the ACT HWDGE queue at iter 9.
 - Block 1's epilogue split: i 128..239 are finished right after tile
   14 (off the critical path, stored at the end in parallel with the
   tail), so only the last 16 i's epilogue follows the final DMA.
"""

import numpy as np

import concourse.bass as bass
import concourse.bacc as bacc
import concourse.tile as tile
from concourse import mybir
from concourse.bass_utils import run_bass_kernel_spmd
from concourse.masks import make_identity

B, N, H = 4, 512, 128
P = 128
JJ = N // P           # 4 consecutive j's per partition
ISUB = 16             # i's per mask tile -> 4 MiB DMAs
IH = N // 2           # 256 i's per core
N_CORES = 8
LN_EPS = 1e-5
F32 = mybir.dt.float32
BF16 = mybir.dt.bfloat16


def build_nc(ih=IH):
    nc = bacc.Bacc()
    s_d = nc.declare_dram_parameter("s", [N, H], F32, isOutput=False)
    w_d = nc.declare_dram_parameter("w", [H, H], F32, isOutput=False)
    b_d = nc.declare_dram_parameter("b", [H], F32, isOutput=False)
    mask_d = nc.declare_dram_parameter("mask", [ih, N, H], F32, isOutput=False)
    out_d = nc.declare_dram_parameter("out", [ih, H], F32, isOutput=True)

    nit = ih // ISUB
    # Full HW config gets the hand-tuned mid-stream epilogues; other ih
    # (CoreSim runs) use the generic end epilogue.
    full = ih == 2 * P and nit == 16

    with tile.TileContext(nc) as tc:
        with (
            tc.tile_pool(name="consts", bufs=1) as consts,
            tc.tile_pool(name="small", bufs=4) as small,
            tc.tile_pool(name="loads", bufs=4) as loads,
            tc.tile_pool(name="prods", bufs=3) as prods,
            tc.tile_pool(name="outs", bufs=3) as outs,
        ):
            stage1_psum = tc.tile_pool(name="spsum", bufs=1, space="PSUM")
            spsum = stage1_psum.__enter__()
            # ---------------- constants ----------------
            # gpsimd-produced constants all precede make_identity so the
            # single carrier wait (Pool sem) covers every one of them.
            ones_col = consts.tile([P, 1], BF16)
            nc.gpsimd.memset(ones_col, 1.0)
            ones_row = consts.tile([1, P], F32)
            nc.gpsimd.memset(ones_row, 1.0)
            # eps on DVE: its consumer (ACT Sqrt) already waits on DVE
            # for mv, and one DVE sem wait covers both.
            eps_t = consts.tile([P, 1], F32)
            nc.vector.memset(eps_t, LN_EPS)
            ident = consts.tile([P, P], F32)
            make_identity(nc, ident)

            # ---- parameter loads, at the HWDGE queue heads ----
            w_sb = consts.tile([H, H], F32)
            nc.sync.dma_start(out=w_sb, in_=w_d[:, :])
            bias_sb = consts.tile([1, H], F32)
            b_ap = b_d[:]
            bias_src = bass.AP(
                tensor=b_ap.tensor, offset=b_ap.offset, ap=[[0, 1]] + list(b_ap.ap)
            )
            nc.scalar.dma_start(out=bias_sb, in_=bias_src)
            # s rows gathered so partition p of block jj holds node
            # j = JJ*p + jj -- m lands directly in the m_perm layout.
            s_full = s_d[:, :]
            s_sbs = []
            for jj in range(JJ):
                s_sb = small.tile([P, H], F32, tag=f"s_sb{jj}")
                s_src = bass.AP(
                    tensor=s_full.tensor,
                    offset=jj * H,
                    ap=[[JJ * H, P], [1, H]],
                )
                deng = nc.sync if jj % 2 == 0 else nc.scalar
                deng.dma_start(out=s_sb, in_=s_src)
                s_sbs.append(s_sb)

            # Wait-carrier: walrus allows only ONE sync wait per
            # Matmult, so absorb the gpsimd(identity) dependency into a
            # throwaway PE op; later matmuls then only carry their own
            # single DMA/engine wait.
            carrier_ps = spsum.tile([P, P], F32)
            nc.tensor.transpose(carrier_ps, ident, ident)

            # W^T via PE-transpose: (o,h) -> (h,o)
            wT_ps = spsum.tile([H, H], F32)
            nc.tensor.transpose(wT_ps, w_sb, ident)
            wT_sb = consts.tile([H, H], F32)
            nc.scalar.copy(wT_sb, wT_ps)

            # ------------- m = SiLU(LN(s @ W.T + b)) -------------
            sT_all = spsum.tile([P, JJ * P], F32)
            h_all = spsum.tile([P, JJ * H], F32)
            for jj in range(JJ):
                nc.tensor.matmul(
                    sT_all[:, jj * P:(jj + 1) * P],
                    lhsT=s_sbs[jj],
                    rhs=ident,
                    is_transpose=True,
                    start=(jj == 0),
                    stop=(jj == JJ - 1),
                )
            sT_sb = consts.tile([P, JJ * P], F32)
            nc.scalar.copy(sT_sb, sT_all)
            for jj in range(JJ):
                nc.tensor.matmul(
                    h_all[:, jj * H:(jj + 1) * H],
                    lhsT=sT_sb[:, jj * P:(jj + 1) * P],
                    rhs=wT_sb,
                    start=(jj == 0),
                    stop=False,
                )
                nc.tensor.matmul(
                    h_all[:, jj * H:(jj + 1) * H],
                    lhsT=ones_row,
                    rhs=bias_sb,
                    start=False,
                    stop=(jj == JJ - 1),
                )

            # m_perm[p, jj, :] = m[JJ*p + jj, :]
            # Sqrt and Sigmoid phases in separate jj-loops: alternating
            # them per jj makes ACT reload its 16 KiB function table
            # (1.3us each), serializing the m_perm critical path.
            m_perm = consts.tile([P, JJ, H], F32)
            xcs, stdvs, xns = [], [], []
            for jj in range(JJ):
                h_ps = h_all[:, jj * H:(jj + 1) * H]
                stats = small.tile([P, 6], F32, tag=f"stats{jj}")
                nc.vector.bn_stats(stats, h_ps)
                mv = small.tile([P, 2], F32, tag=f"mv{jj}")
                nc.vector.bn_aggr(mv, stats)
                xc = small.tile([P, H], F32, tag=f"xc{jj}")
                nc.vector.tensor_scalar_sub(xc, h_ps, mv[:, 0:1])
                stdv = small.tile([P, 1], F32, tag=f"stdv{jj}")
                nc.scalar.activation(
                    stdv, mv[:, 1:2], mybir.ActivationFunctionType.Sqrt, bias=eps_t
                )
                xcs.append(xc)
                stdvs.append(stdv)
            for jj in range(JJ):
                rstd = small.tile([P, 1], F32, tag=f"rstd{jj}")
                nc.vector.reciprocal(rstd, stdvs[jj])
                xn = small.tile([P, H], F32, tag=f"xn{jj}")
                nc.vector.tensor_scalar_mul(xn, xcs[jj], rstd)
                xns.append(xn)
            for jj in range(JJ):
                sg = small.tile([P, H], F32, tag=f"sg{jj}")
                nc.scalar.activation(
                    sg, xns[jj], mybir.ActivationFunctionType.Sigmoid
                )
                nc.vector.tensor_mul(m_perm[:, jj, :], xns[jj], sg)

            # m broadcast along the i axis: 0-stride free axis.
            def m_bcast(n):
                return bass.AP(
                    tensor=m_perm.tensor,
                    offset=m_perm.offset,
                    ap=[list(m_perm.ap[0]), [0, n]]
                    + [list(x) for x in m_perm.ap[1:]],
                )

            # stage-1 PSUM pools stay open: releasing them would put a
            # (PE+DVE) release-wait on stage-2's first Matmult, which
            # walrus cannot encode.
            # ------- out[i,h] = sum_j mask[i,j,h] * m[j,h] -------
            # acc2[h, i] += pt[:, ii, jj, :].T @ ones  (partition-reduce
            # over p via PE, free-axis reduce over jj via PSUM accum).
            opsum_cm = tc.tile_pool(name="opsum", bufs=1, space="PSUM")
            opsum = opsum_cm.__enter__()
            tpsum_cm = tc.tile_pool(name="tpsum", bufs=2, space="PSUM")
            tpsum = tpsum_cm.__enter__()
            acc2 = opsum.tile([P, ih], F32)

            def epi_compute(tag, i0, w):
                """acc2[:, i0:i0+w] -> oT [w, H] in SBUF (DVE+PE only)."""
                accT = outs.tile([P, w], F32, tag="accT", name=f"accT{tag}",
                                 bufs=2)
                nc.vector.tensor_copy(accT, acc2[:, i0:i0 + w])
                tp = tpsum.tile([w, P], F32, tag="tp", name=f"tp{tag}")
                nc.tensor.transpose(tp, accT, ident)
                oT = outs.tile([w, P], F32, tag="oT", name=f"oT{tag}", bufs=2)
                nc.vector.tensor_copy(oT, tp)
                return oT

            oT0 = oTA = None
            for it in range(nit):
                # Mid-stream block-0 store: emitted at the top of iter 9
                # so the ACT engine reaches the (long-satisfied) oT0 sem
                # wait without stalling later mask triggers.
                if full and it == 9:
                    nc.scalar.dma_start(out=out_d[0:P, :], in_=oT0)
                mt = loads.tile([P, ISUB, JJ, H], F32, tag="mt", name=f"mt{it}")
                # Two 2 MiB halves on opposite HWDGE queues: tiles then
                # complete sequentially (~11.6us apart) instead of in
                # near-simultaneous pairs, which keeps the DVE multiply
                # backlog near zero through the stream.  The final three
                # tiles use 1 MiB quarters so the post-stream tail is a
                # single 2.3us quarter-multiply.
                last = it >= nit - 3 and nit > 3
                nsub = 4 if last else (2 if nit > 1 else 1)
                sub = ISUB // nsub
                for q in range(nsub):
                    src = mask_d[
                        it * ISUB + q * sub:it * ISUB + (q + 1) * sub, :, :
                    ].rearrange("i (p jj) h -> p i jj h", jj=JJ)
                    # Alternate the two HWDGE queues (SP / Activation)
                    # so descriptor-gen + completion latency of one
                    # queue hides behind the other's transfers.
                    deng = nc.sync if (it + q) % 2 == 0 else nc.scalar
                    deng.dma_start(out=mt[:, q * sub:(q + 1) * sub], in_=src)
                    pt = prods.tile([P, sub, JJ, H], BF16, tag=f"pt{sub}",
                                    name=f"pt{it}_{q}",
                                    bufs=2)
                    # Alternate the elementwise multiply between DVE and
                    # the Pool engine: DVE alone is ~92% occupied by the
                    # multiplies (121 G elem/s, ~20% slower when racing
                    # the DMA writes into the same tile), so any stall
                    # accumulates into a post-stream tail.
                    meng = nc.vector if (it * nsub + q) % 2 == 0 else nc.gpsimd
                    meng.tensor_mul(
                        pt, mt[:, q * sub:(q + 1) * sub], m_bcast(sub)
                    )
                    for ii in range(sub):
                        i = it * ISUB + q * sub + ii
                        for jj in range(JJ):
                            # One accumulation group spans the whole
                            # bank: start zeroes the full zero region,
                            # so only the global first/last matmuls
                            # carry start/stop.
                            nc.tensor.matmul(
                                acc2[:, i:i + 1],
                                lhsT=pt[:, ii, jj, :],
                                rhs=ones_col,
                                start=(i == 0 and jj == 0),
                                stop=(i == ih - 1 and jj == JJ - 1),
                            )
                # Block 0 (i 0..127) is fully accumulated after tile 7:
                # run its epilogue mid-stream using ONLY DVE + PE.
                if full and it == 7:
                    oT0 = epi_compute("0", 0, P)
                # i 128..239 fully accumulated after tile 14: compute
                # their epilogue off the critical path; store at end.
                if full and it == nit - 2:
                    oTA = epi_compute("A", P, ih - P - ISUB)

            if full:
                # A-store drains right after the last sync-queue mask
                # descriptors; B (the last 16 i's) is the only epilogue
                # on the post-stream critical path.
                nc.sync.dma_start(out=out_d[P:ih - ISUB, :], in_=oTA)
                oTB = epi_compute("B", ih - ISUB, ISUB)
                nc.scalar.dma_start(out=out_d[ih - ISUB:ih, :], in_=oTB)
            else:
                nblk = (ih + P - 1) // P
                for blk in range(nblk):
                    w = min(P, ih - blk * P)
                    oT = epi_compute(f"g{blk}", blk * P, w)
                    deng = nc.sync if blk % 2 == 0 else nc.scalar
                    deng.dma_start(out=out_d[blk * P:blk * P + w, :], in_=oT)
            tpsum_cm.__exit__(None, None, None)
            opsum_cm.__exit__(None, None, None)
            stage1_psum.__exit__(None, None, None)
    nc.finalize()
    return nc


_NC_CACHE = {}


def _get_nc():
    key = "main"
    if key not in _NC_CACHE:
        _NC_CACHE[key] = build_nc()
    return _NC_CACHE[key]


def kernel(s, ef_mask, W, b):
    s = np.ascontiguousarray(s, dtype=np.float32)
    ef_mask = np.ascontiguousarray(ef_mask, dtype=np.float32)
    W = np.ascontiguousarray(W, dtype=np.float32)
    b = np.ascontiguousarray(b, dtype=np.float32)

    nc = _get_nc()
    in_maps = []
    for c in range(N_CORES):
        bb = c // 2
        half = c % 2
        in_maps.append(
            {
                "s": s[bb],
                "w": W,
                "b": b,
                "mask": ef_mask[bb, half * IH:(half + 1) * IH],
            }
        )
    res = run_bass_kernel_spmd(nc, in_maps, list(range(N_CORES))).results
    out = np.empty((B, N, H), dtype=np.float32)
    for c in range(N_CORES):
        bb = c // 2
        half = c % 2
        out[bb, half * IH:(half + 1) * IH] = res[c]["out"]
    return out


# revision 10
# speedup vs baseline: 1.1822x; 1.1822x over previous
"""Trainium2 Bass kernel for DirCFConv-style GNN message passing, v4.

Computes, for inputs s:(B,N,H) f32, ef_mask:(B,N,N,H) f32, W:(H,H), b:(H,):
    m   = SiLU(LayerNorm(s @ W.T + b))          # (B,N,H)
    out[b,i,h] = sum_j ef_mask[b,i,j,h] * m[b,j,h]

Sharding: 8 cores, core c handles batch b = c // 2 and query-node half
i in [ (c%2)*256, (c%2)*256+256 ).  Each core streams its 64 MiB mask
shard from HBM (the roofline: 64 MiB at the 358 GB/s per-core cap is
187 us; v2's stream already ran at that rate from t=8.6..196.5us).

v2 layout (kept): SBUF partition p holds the JJ=4 consecutive j's
{4p..4p+3}, so each DMA descriptor covers a contiguous (jj,h) run of
2 KiB.  Every tile's DMA is issued as two 2 MiB halves on opposite
HWDGE queues (SP/ACT); the last three tiles use 1 MiB quarters.  The
multiply uses a partition-permuted m broadcast along i with a 0-stride
AP; the j-reduction is JJ accumulating PE matmuls per query node on a
bf16 product.  The in-loop trigger emission and queue parity are
LOAD-BEARING: prefetch-style reorderings (all triggers upfront, or a
deep single-queue prefetch) stall the issuing engine on the ~2-chunk
HWDGE descriptor ring and/or unbalance the queues, dropping the
steady-state stream from 357 GB/s to ~320 (measured 227us and 238us).

v4 changes vs v2 (222.8us):
 - s/W/b loads moved from the gpsimd SWDGE queue (slow software
   descriptors; m was only ready at t=32.7us, leaving a DVE backlog
   that ran 20us past the final DMA) to the heads of the two HWDGE
   queues, in front of the mask triggers.  Stage 1 now finishes before
   tile 0 lands and the DVE tracks the stream with no backlog.
 - Mid-stream block-0 output store moved from SWDGE (928ns per 512B
   descriptor, a 15us trickle) to the ACT HWDGE queue at iter 9.
 - Block 1's epilogue split: i 128..239 are finished right after tile
   14 (off the critical path, stored at the end in parallel with the
   tail), so only the last 16 i's epilogue follows the final DMA.
"""

import numpy as np

import concourse.bass as bass
import concourse.bacc as bacc
import concourse.tile as tile
from concourse import mybir
from concourse.bass_utils import run_bass_kernel_spmd
from concourse.masks import make_identity

B, N, H = 4, 512, 128
P = 128
JJ = N // P           # 4 consecutive j's per partition
ISUB = 16             # i's per mask tile -> 4 MiB DMAs
IH = N // 2           # 256 i's per core
N_CORES = 8
LN_EPS = 1e-5
F32 = mybir.dt.float32
BF16 = mybir.dt.bfloat16


def build_nc(ih=IH):
    nc = bacc.Bacc()
    s_d = nc.declare_dram_parameter("s", [N, H], F32, isOutput=False)
    w_d = nc.declare_dram_parameter("w", [H, H], F32, isOutput=False)
    b_d = nc.declare_dram_parameter("b", [H], F32, isOutput=False)
    mask_d = nc.declare_dram_parameter("mask", [ih, N, H], F32, isOutput=False)
    out_d = nc.declare_dram_parameter("out", [ih, H], F32, isOutput=True)

    nit = ih // ISUB
    # Full HW config gets the hand-tuned mid-stream epilogues; other ih
    # (CoreSim runs) use the generic end epilogue.
    full = ih == 2 * P and nit == 16

    with tile.TileContext(nc) as tc:
        with (
            tc.tile_pool(name="consts", bufs=1) as consts,
            tc.tile_pool(name="small", bufs=4) as small,
            tc.tile_pool(name="loads", bufs=4) as loads,
            tc.tile_pool(name="prods", bufs=3) as prods,
            tc.tile_pool(name="outs", bufs=3) as outs,
        ):
            stage1_psum = tc.tile_pool(name="spsum", bufs=1, space="PSUM")
            spsum = stage1_psum.__enter__()
            # ---------------- constants ----------------
            # gpsimd-produced constants all precede make_identity so the
            # single carrier wait (Pool sem) covers every one of them.
            ones_col = consts.tile([P, 1], BF16)
            nc.gpsimd.memset(ones_col, 1.0)
            ones_row = consts.tile([1, JJ * P], F32)
            nc.gpsimd.memset(ones_row, 1.0)
            # eps on DVE: its consumer (ACT Sqrt) already waits on DVE
            # for mv, and one DVE sem wait covers both.
            eps_t = consts.tile([P, 1], F32)
            nc.vector.memset(eps_t, LN_EPS)
            ident = consts.tile([P, P], F32)
            make_identity(nc, ident)

            # ---- parameter loads, at the HWDGE queue heads ----
            w_sb = consts.tile([H, H], F32)
            nc.sync.dma_start(out=w_sb, in_=w_d[:, :])
            bias_sb = consts.tile([1, H], F32)
            b_ap = b_d[:]
            bias_src = bass.AP(
                tensor=b_ap.tensor, offset=b_ap.offset, ap=[[0, 1]] + list(b_ap.ap)
            )
            nc.scalar.dma_start(out=bias_sb, in_=bias_src)
            # s rows gathered so partition p of block jj holds node
            # j = JJ*p + jj -- m lands directly in the m_perm layout.
            s_full = s_d[:, :]
            s_sbs = []
            for jj in range(JJ):
                s_sb = small.tile([P, H], F32, tag=f"s_sb{jj}")
                s_src = bass.AP(
                    tensor=s_full.tensor,
                    offset=jj * H,
                    ap=[[JJ * H, P], [1, H]],
                )
                deng = nc.sync if jj % 2 == 0 else nc.scalar
                deng.dma_start(out=s_sb, in_=s_src)
                s_sbs.append(s_sb)

            # Wait-carrier: walrus allows only ONE sync wait per
            # Matmult, so absorb the gpsimd(identity) dependency into a
            # throwaway PE op; later matmuls then only carry their own
            # single DMA/engine wait.
            carrier_ps = spsum.tile([P, P], F32)
            nc.tensor.transpose(carrier_ps, ident, ident)

            # W^T via PE-transpose: (o,h) -> (h,o)
            wT_ps = spsum.tile([H, H], F32)
            nc.tensor.transpose(wT_ps, w_sb, ident)
            wT_sb = consts.tile([H, H], F32)
            nc.scalar.copy(wT_sb, wT_ps)

            # ------------- m = SiLU(LN(s @ W.T + b)) -------------
            # hT[o, j] = sum_h W[o,h] sT[h,j] + b[o]: ONE wT-stationary
            # matmul over 512 moving columns instead of eight
            # sT-stationary f32 matmuls (f32 LDWEIGHTS is 2-pass at 4x
            # row cost; the old chain held m back to t=28, an ~8us DVE
            # deficit that persisted to the end of the stream).  Then
            # transpose hT back to [j, o] blocks for the free-axis LN.
            sT_all = spsum.tile([P, JJ * P], F32)
            h_all = spsum.tile([P, JJ * H], F32)
            hT_ps = spsum.tile([P, JJ * P], F32)
            for jj in range(JJ):
                nc.tensor.matmul(
                    sT_all[:, jj * P:(jj + 1) * P],
                    lhsT=s_sbs[jj],
                    rhs=ident,
                    is_transpose=True,
                    start=(jj == 0),
                    stop=(jj == JJ - 1),
                )
            sT_sb = consts.tile([P, JJ * P], F32)
            nc.scalar.copy(sT_sb, sT_all)
            nc.tensor.matmul(
                hT_ps, lhsT=wT_sb, rhs=sT_sb, start=True, stop=False
            )
            nc.tensor.matmul(
                hT_ps, lhsT=bias_sb, rhs=ones_row, start=False, stop=True
            )
            hT_sb = consts.tile([P, JJ * P], F32)
            nc.vector.tensor_copy(hT_sb, hT_ps)
            for jj in range(JJ):
                nc.tensor.matmul(
                    h_all[:, jj * H:(jj + 1) * H],
                    lhsT=hT_sb[:, jj * P:(jj + 1) * P],
                    rhs=ident,
                    is_transpose=True,
                    start=(jj == 0),
                    stop=(jj == JJ - 1),
                )

            # m_perm[p, jj, :] = m[JJ*p + jj, :]
            # Sqrt and Sigmoid phases in separate jj-loops: alternating
            # them per jj makes ACT reload its 16 KiB function table
            # (1.3us each), serializing the m_perm critical path.
            m_perm = consts.tile([P, JJ, H], F32)
            xcs, stdvs, xns = [], [], []
            for jj in range(JJ):
                h_ps = h_all[:, jj * H:(jj + 1) * H]
                stats = small.tile([P, 6], F32, tag=f"stats{jj}")
                nc.vector.bn_stats(stats, h_ps)
                mv = small.tile([P, 2], F32, tag=f"mv{jj}")
                nc.vector.bn_aggr(mv, stats)
                xc = small.tile([P, H], F32, tag=f"xc{jj}")
                nc.vector.tensor_scalar_sub(xc, h_ps, mv[:, 0:1])
                stdv = small.tile([P, 1], F32, tag=f"stdv{jj}")
                nc.scalar.activation(
                    stdv, mv[:, 1:2], mybir.ActivationFunctionType.Sqrt, bias=eps_t
                )
                xcs.append(xc)
                stdvs.append(stdv)
            for jj in range(JJ):
                rstd = small.tile([P, 1], F32, tag=f"rstd{jj}")
                nc.vector.reciprocal(rstd, stdvs[jj])
                xn = small.tile([P, H], F32, tag=f"xn{jj}")
                nc.vector.tensor_scalar_mul(xn, xcs[jj], rstd)
                xns.append(xn)
            for jj in range(JJ):
                sg = small.tile([P, H], F32, tag=f"sg{jj}")
                nc.scalar.activation(
                    sg, xns[jj], mybir.ActivationFunctionType.Sigmoid
                )
                nc.vector.tensor_mul(m_perm[:, jj, :], xns[jj], sg)

            # m broadcast along the i axis: 0-stride free axis.
            def m_bcast(n):
                return bass.AP(
                    tensor=m_perm.tensor,
                    offset=m_perm.offset,
                    ap=[list(m_perm.ap[0]), [0, n]]
                    + [list(x) for x in m_perm.ap[1:]],
                )

            # stage-1 PSUM pools stay open: releasing them would put a
            # (PE+DVE) release-wait on stage-2's first Matmult, which
            # walrus cannot encode.
            # ------- out[i,h] = sum_j mask[i,j,h] * m[j,h] -------
            # acc2[h, i] += pt[:, ii, jj, :].T @ ones  (partition-reduce
            # over p via PE, free-axis reduce over jj via PSUM accum).
            opsum_cm = tc.tile_pool(name="opsum", bufs=1, space="PSUM")
            opsum = opsum_cm.__enter__()
            tpsum_cm = tc.tile_pool(name="tpsum", bufs=2, space="PSUM")
            tpsum = tpsum_cm.__enter__()
            acc2 = opsum.tile([P, ih], F32)

            def epi_compute(tag, i0, w):
                """acc2[:, i0:i0+w] -> oT [w, H] in SBUF (DVE+PE only)."""
                accT = outs.tile([P, w], F32, tag="accT", name=f"accT{tag}",
                                 bufs=2)
                nc.vector.tensor_copy(accT, acc2[:, i0:i0 + w])
                tp = tpsum.tile([w, P], F32, tag="tp", name=f"tp{tag}")
                nc.tensor.transpose(tp, accT, ident)
                oT = outs.tile([w, P], F32, tag="oT", name=f"oT{tag}", bufs=2)
                nc.vector.tensor_copy(oT, tp)
                return oT

            oT0 = oTA = None
            for it in range(nit):
                # Mid-stream block-0 store: emitted at the top of iter 9
                # so the ACT engine reaches the (long-satisfied) oT0 sem
                # wait without stalling later mask triggers.
                if full and it == 9:
                    nc.scalar.dma_start(out=out_d[0:P, :], in_=oT0)
                mt = loads.tile([P, ISUB, JJ, H], F32, tag="mt", name=f"mt{it}")
                # Two 2 MiB halves on opposite HWDGE queues: tiles then
                # complete sequentially (~11.6us apart) instead of in
                # near-simultaneous pairs, which keeps the DVE multiply
                # backlog near zero through the stream.  The final three
                # tiles use 1 MiB quarters so the post-stream tail is a
                # single 2.3us quarter-multiply.
                last = it >= nit - 3 and nit > 3
                nsub = 4 if last else (2 if nit > 1 else 1)
                sub = ISUB // nsub
                for q in range(nsub):
                    src = mask_d[
                        it * ISUB + q * sub:it * ISUB + (q + 1) * sub, :, :
                    ].rearrange("i (p jj) h -> p i jj h", jj=JJ)
                    # Alternate the two HWDGE queues (SP / Activation)
                    # so descriptor-gen + completion latency of one
                    # queue hides behind the other's transfers.
                    deng = nc.sync if (it + q) % 2 == 0 else nc.scalar
                    deng.dma_start(out=mt[:, q * sub:(q + 1) * sub], in_=src)
                    pt = prods.tile([P, sub, JJ, H], BF16, tag=f"pt{sub}",
                                    name=f"pt{it}_{q}",
                                    bufs=2)
                    # The multiply stays on DVE alone: offloading
                    # alternate chunks to the Pool engine stretched BOTH
                    # engines ~2x (SBUF bandwidth contention, measured
                    # 265us), so concurrency there is a net loss.
                    nc.vector.tensor_mul(
                        pt, mt[:, q * sub:(q + 1) * sub], m_bcast(sub)
                    )
                    for ii in range(sub):
                        i = it * ISUB + q * sub + ii
                        for jj in range(JJ):
                            # One accumulation group spans the whole
                            # bank: start zeroes the full zero region,
                            # so only the global first/last matmuls
                            # carry start/stop.
                            nc.tensor.matmul(
                                acc2[:, i:i + 1],
                                lhsT=pt[:, ii, jj, :],
                                rhs=ones_col,
                                start=(i == 0 and jj == 0),
                                stop=(i == ih - 1 and jj == JJ - 1),
                            )
                # Block 0 (i 0..127) is fully accumulated after tile 7:
                # run its epilogue mid-stream using ONLY DVE + PE.
                if full and it == 7:
                    oT0 = epi_compute("0", 0, P)
                # i 128..239 fully accumulated after tile 14: compute
                # their epilogue off the critical path; store at end.
                if full and it == nit - 2:
                    oTA = epi_compute("A", P, ih - P - ISUB)

            if full:
                # A-store drains right after the last sync-queue mask
                # descriptors; B (the last 16 i's) is the only epilogue
                # on the post-stream critical path.
                nc.sync.dma_start(out=out_d[P:ih - ISUB, :], in_=oTA)
                oTB = epi_compute("B", ih - ISUB, ISUB)
                nc.scalar.dma_start(out=out_d[ih - ISUB:ih, :], in_=oTB)
            else:
                nblk = (ih + P - 1) // P
                for blk in range(nblk):
                    w = min(P, ih - blk * P)
                    oT = epi_compute(f"g{blk}", blk * P, w)
                    deng = nc.sync if blk % 2 == 0 else nc.scalar
                    deng.dma_start(out=out_d[blk * P:blk * P + w, :], in_=oT)
            tpsum_cm.__exit__(None, None, None)
            opsum_cm.__exit__(None, None, None)
            stage1_psum.__exit__(None, None, None)
    nc.finalize()
    return nc


_NC_CACHE = {}


def _get_nc():
    key = "main"
    if key not in _NC_CACHE:
        _NC_CACHE[key] = build_nc()
    return _NC_CACHE[key]


def kernel(s, ef_mask, W, b):
    s = np.ascontiguousarray(s, dtype=np.float32)
    ef_mask = np.ascontiguousarray(ef_mask, dtype=np.float32)
    W = np.ascontiguousarray(W, dtype=np.float32)
    b = np.ascontiguousarray(b, dtype=np.float32)

    nc = _get_nc()
    in_maps = []
    for c in range(N_CORES):
        bb = c // 2
        half = c % 2
        in_maps.append(
            {
                "s": s[bb],
                "w": W,
                "b": b,
                "mask": ef_mask[bb, half * IH:(half + 1) * IH],
            }
        )
    res = run_bass_kernel_spmd(nc, in_maps, list(range(N_CORES))).results
    out = np.empty((B, N, H), dtype=np.float32)
    for c in range(N_CORES):
        bb = c // 2
        half = c % 2
        out[bb, half * IH:(half + 1) * IH] = res[c]["out"]
    return out


# revision 12
# speedup vs baseline: 1.2720x; 1.0760x over previous
"""Trainium2 Bass kernel for DirCFConv-style GNN message passing, v4.

Computes, for inputs s:(B,N,H) f32, ef_mask:(B,N,N,H) f32, W:(H,H), b:(H,):
    m   = SiLU(LayerNorm(s @ W.T + b))          # (B,N,H)
    out[b,i,h] = sum_j ef_mask[b,i,j,h] * m[b,j,h]

Sharding: 8 cores, core c handles batch b = c // 2 and query-node half
i in [ (c%2)*256, (c%2)*256+256 ).  Each core streams its 64 MiB mask
shard from HBM (the roofline: 64 MiB at the 358 GB/s per-core cap is
187 us; v2's stream already ran at that rate from t=8.6..196.5us).

v2 layout (kept): SBUF partition p holds the JJ=4 consecutive j's
{4p..4p+3}, so each DMA descriptor covers a contiguous (jj,h) run of
2 KiB.  Every tile's DMA is issued as two 2 MiB halves on opposite
HWDGE queues (SP/ACT); the last three tiles use 1 MiB quarters.  The
multiply uses a partition-permuted m broadcast along i with a 0-stride
AP; the j-reduction is JJ accumulating PE matmuls per query node on a
bf16 product.  The in-loop trigger emission and queue parity are
LOAD-BEARING: prefetch-style reorderings (all triggers upfront, or a
deep single-queue prefetch) stall the issuing engine on the ~2-chunk
HWDGE descriptor ring and/or unbalance the queues, dropping the
steady-state stream from 357 GB/s to ~320 (measured 227us and 238us).

v4 changes vs v2 (222.8us):
 - s/W/b loads moved from the gpsimd SWDGE queue (slow software
   descriptors; m was only ready at t=32.7us, leaving a DVE backlog
   that ran 20us past the final DMA) to the heads of the two HWDGE
   queues, in front of the mask triggers.  Stage 1 now finishes before
   tile 0 lands and the DVE tracks the stream with no backlog.
 - Mid-stream block-0 output store moved from SWDGE (928ns per 512B
   descriptor, a 15us trickle) to the ACT HWDGE queue at iter 9.
 - Block 1's epilogue split: i 128..239 are finished right after tile
   14 (off the critical path, stored at the end in parallel with the
   tail), so only the last 16 i's epilogue follows the final DMA.
"""

import numpy as np

import concourse.bass as bass
import concourse.bacc as bacc
import concourse.tile as tile
from concourse import mybir
from concourse.bass_utils import run_bass_kernel_spmd
from concourse.masks import make_identity

B, N, H = 4, 512, 128
P = 128
JJ = N // P           # 4 consecutive j's per partition
ISUB = 16             # i's per mask tile -> 4 MiB DMAs
IH = N // 2           # 256 i's per core
N_CORES = 8
LN_EPS = 1e-5
F32 = mybir.dt.float32
BF16 = mybir.dt.bfloat16


def build_nc(ih=IH):
    nc = bacc.Bacc()
    s_d = nc.declare_dram_parameter("s", [N, H], F32, isOutput=False)
    w_d = nc.declare_dram_parameter("w", [H, H], F32, isOutput=False)
    b_d = nc.declare_dram_parameter("b", [H], F32, isOutput=False)
    mask_d = nc.declare_dram_parameter("mask", [ih, N, H], F32, isOutput=False)
    out_d = nc.declare_dram_parameter("out", [ih, H], F32, isOutput=True)

    nit = ih // ISUB
    # Full HW config gets the hand-tuned mid-stream epilogues; other ih
    # (CoreSim runs) use the generic end epilogue.
    full = ih == 2 * P and nit == 16

    with tile.TileContext(nc) as tc:
        with (
            tc.tile_pool(name="consts", bufs=1) as consts,
            tc.tile_pool(name="small", bufs=4) as small,
            tc.tile_pool(name="loads", bufs=4) as loads,
            tc.tile_pool(name="prods", bufs=3) as prods,
            tc.tile_pool(name="outs", bufs=3) as outs,
        ):
            stage1_psum = tc.tile_pool(name="spsum", bufs=1, space="PSUM")
            spsum = stage1_psum.__enter__()
            # ---------------- constants ----------------
            # gpsimd-produced constants all precede make_identity so the
            # single carrier wait (Pool sem) covers every one of them.
            ones_col = consts.tile([P, 1], BF16)
            nc.gpsimd.memset(ones_col, 1.0)
            ones_row = consts.tile([1, JJ * P], F32)
            nc.gpsimd.memset(ones_row, 1.0)
            # eps on DVE: its consumer (ACT Sqrt) already waits on DVE
            # for mv, and one DVE sem wait covers both.
            eps_t = consts.tile([P, 1], F32)
            nc.vector.memset(eps_t, LN_EPS)
            ident = consts.tile([P, P], F32)
            make_identity(nc, ident)

            # ---- parameter loads, at the HWDGE queue heads ----
            w_sb = consts.tile([H, H], F32)
            nc.sync.dma_start(out=w_sb, in_=w_d[:, :])
            bias_sb = consts.tile([1, H], F32)
            b_ap = b_d[:]
            bias_src = bass.AP(
                tensor=b_ap.tensor, offset=b_ap.offset, ap=[[0, 1]] + list(b_ap.ap)
            )
            nc.scalar.dma_start(out=bias_sb, in_=bias_src)
            # s rows gathered so partition p of block jj holds node
            # j = JJ*p + jj -- m lands directly in the m_perm layout.
            s_full = s_d[:, :]
            s_sbs = []
            for jj in range(JJ):
                s_sb = small.tile([P, H], F32, tag=f"s_sb{jj}")
                s_src = bass.AP(
                    tensor=s_full.tensor,
                    offset=jj * H,
                    ap=[[JJ * H, P], [1, H]],
                )
                deng = nc.sync if jj % 2 == 0 else nc.scalar
                deng.dma_start(out=s_sb, in_=s_src)
                s_sbs.append(s_sb)

            # Wait-carrier: walrus allows only ONE sync wait per
            # Matmult, so absorb the gpsimd(identity) dependency into a
            # throwaway PE op; later matmuls then only carry their own
            # single DMA/engine wait.
            carrier_ps = spsum.tile([P, P], F32)
            nc.tensor.transpose(carrier_ps, ident, ident)

            # W^T via PE-transpose: (o,h) -> (h,o)
            wT_ps = spsum.tile([H, H], F32)
            nc.tensor.transpose(wT_ps, w_sb, ident)
            wT_sb = consts.tile([H, H], F32)
            nc.scalar.copy(wT_sb, wT_ps)

            # ------------- m = SiLU(LN(s @ W.T + b)) -------------
            # hT[o, j] = sum_h W[o,h] sT[h,j] + b[o]: ONE wT-stationary
            # matmul over 512 moving columns instead of eight
            # sT-stationary f32 matmuls (f32 LDWEIGHTS is 2-pass at 4x
            # row cost; the old chain held m back to t=28, an ~8us DVE
            # deficit that persisted to the end of the stream).  Then
            # transpose hT back to [j, o] blocks for the free-axis LN.
            sT_all = spsum.tile([P, JJ * P], F32)
            h_all = spsum.tile([P, JJ * H], F32)
            hT_ps = spsum.tile([P, JJ * P], F32)
            for jj in range(JJ):
                nc.tensor.matmul(
                    sT_all[:, jj * P:(jj + 1) * P],
                    lhsT=s_sbs[jj],
                    rhs=ident,
                    is_transpose=True,
                    start=(jj == 0),
                    stop=(jj == JJ - 1),
                )
            sT_sb = consts.tile([P, JJ * P], F32)
            nc.scalar.copy(sT_sb, sT_all)
            nc.tensor.matmul(
                hT_ps, lhsT=wT_sb, rhs=sT_sb, start=True, stop=False
            )
            nc.tensor.matmul(
                hT_ps, lhsT=bias_sb, rhs=ones_row, start=False, stop=True
            )
            hT_sb = consts.tile([P, JJ * P], F32)
            nc.vector.tensor_copy(hT_sb, hT_ps)
            for jj in range(JJ):
                nc.tensor.matmul(
                    h_all[:, jj * H:(jj + 1) * H],
                    lhsT=hT_sb[:, jj * P:(jj + 1) * P],
                    rhs=ident,
                    is_transpose=True,
                    start=(jj == 0),
                    stop=(jj == JJ - 1),
                )

            # m_perm[p, jj, :] = m[JJ*p + jj, :]
            # Sqrt and Sigmoid phases in separate jj-loops: alternating
            # them per jj makes ACT reload its 16 KiB function table
            # (1.3us each), serializing the m_perm critical path.
            m_perm = consts.tile([P, JJ, H], F32)
            xcs, stdvs = [], []
            for jj in range(JJ):
                h_ps = h_all[:, jj * H:(jj + 1) * H]
                stats = small.tile([P, 6], F32, tag=f"stats{jj}")
                nc.vector.bn_stats(stats, h_ps)
                mv = small.tile([P, 2], F32, tag=f"mv{jj}")
                nc.vector.bn_aggr(mv, stats)
                xc = small.tile([P, H], F32, tag=f"xc{jj}")
                nc.vector.tensor_scalar_sub(xc, h_ps, mv[:, 0:1])
                stdv = small.tile([P, 1], F32, tag=f"stdv{jj}")
                nc.scalar.activation(
                    stdv, mv[:, 1:2], mybir.ActivationFunctionType.Sqrt, bias=eps_t
                )
                xcs.append(xc)
                stdvs.append(stdv)
            rstds = []
            for jj in range(JJ):
                rstd = small.tile([P, 1], F32, tag=f"rstd{jj}")
                nc.vector.reciprocal(rstd, stdvs[jj])
                rstds.append(rstd)
            for jj in range(JJ):
                # One fused ACT op: m = silu(xc * rstd).  The old
                # sigmoid + two DVE muls cost ~3us of ACT<->DVE
                # ping-pong on the m critical path.
                nc.scalar.activation(
                    m_perm[:, jj, :], xcs[jj],
                    mybir.ActivationFunctionType.Silu, scale=rstds[jj]
                )

            # m broadcast along the i axis: 0-stride free axis.
            def m_bcast(n):
                return bass.AP(
                    tensor=m_perm.tensor,
                    offset=m_perm.offset,
                    ap=[list(m_perm.ap[0]), [0, n]]
                    + [list(x) for x in m_perm.ap[1:]],
                )

            # stage-1 PSUM pools stay open: releasing them would put a
            # (PE+DVE) release-wait on stage-2's first Matmult, which
            # walrus cannot encode.
            # ------- out[i,h] = sum_j mask[i,j,h] * m[j,h] -------
            # acc2[h, i] += pt[:, ii, jj, :].T @ ones  (partition-reduce
            # over p via PE, free-axis reduce over jj via PSUM accum).
            opsum_cm = tc.tile_pool(name="opsum", bufs=1, space="PSUM")
            opsum = opsum_cm.__enter__()
            tpsum_cm = tc.tile_pool(name="tpsum", bufs=2, space="PSUM")
            tpsum = tpsum_cm.__enter__()
            acc2 = opsum.tile([P, ih], F32)

            def epi_compute(tag, i0, w):
                """acc2[:, i0:i0+w] -> oT [w, H] in SBUF (DVE+PE only)."""
                accT = outs.tile([P, w], F32, tag="accT", name=f"accT{tag}",
                                 bufs=2)
                nc.vector.tensor_copy(accT, acc2[:, i0:i0 + w])
                tp = tpsum.tile([w, P], F32, tag="tp", name=f"tp{tag}")
                nc.tensor.transpose(tp, accT, ident)
                oT = outs.tile([w, P], F32, tag="oT", name=f"oT{tag}", bufs=2)
                nc.vector.tensor_copy(oT, tp)
                return oT

            oT0 = oTA = None
            for it in range(nit):
                # Mid-stream block-0 store: emitted at the top of iter 9
                # so the ACT engine reaches the (long-satisfied) oT0 sem
                # wait without stalling later mask triggers.
                if full and it == 9:
                    nc.scalar.dma_start(out=out_d[0:P, :], in_=oT0)
                mt = loads.tile([P, ISUB, JJ, H], F32, tag="mt", name=f"mt{it}")
                # Two 2 MiB halves on opposite HWDGE queues: tiles then
                # complete sequentially (~11.6us apart) instead of in
                # near-simultaneous pairs, which keeps the DVE multiply
                # backlog near zero through the stream.  The final three
                # tiles use 1 MiB quarters so the post-stream tail is a
                # single 2.3us quarter-multiply.
                last = it >= nit - 3 and nit > 3
                nsub = 4 if last else (2 if nit > 1 else 1)
                sub = ISUB // nsub
                for q in range(nsub):
                    src = mask_d[
                        it * ISUB + q * sub:it * ISUB + (q + 1) * sub, :, :
                    ].rearrange("i (p jj) h -> p i jj h", jj=JJ)
                    # Alternate the two HWDGE queues (SP / Activation)
                    # so descriptor-gen + completion latency of one
                    # queue hides behind the other's transfers.
                    deng = nc.sync if (it + q) % 2 == 0 else nc.scalar
                    deng.dma_start(out=mt[:, q * sub:(q + 1) * sub], in_=src)
                    pt = prods.tile([P, sub, JJ, H], BF16, tag=f"pt{sub}",
                                    name=f"pt{it}_{q}",
                                    bufs=2)
                    # The multiply stays on DVE alone: offloading
                    # alternate chunks to the Pool engine stretched BOTH
                    # engines ~2x (SBUF bandwidth contention, measured
                    # 265us), so concurrency there is a net loss.
                    nc.vector.tensor_mul(
                        pt, mt[:, q * sub:(q + 1) * sub], m_bcast(sub)
                    )
                    for ii in range(sub):
                        i = it * ISUB + q * sub + ii
                        for jj in range(JJ):
                            # One accumulation group spans the whole
                            # bank: start zeroes the full zero region,
                            # so only the global first/last matmuls
                            # carry start/stop.
                            nc.tensor.matmul(
                                acc2[:, i:i + 1],
                                lhsT=pt[:, ii, jj, :],
                                rhs=ones_col,
                                start=(i == 0 and jj == 0),
                                stop=(i == ih - 1 and jj == JJ - 1),
                            )
                # Block 0 (i 0..127) is fully accumulated after tile 7:
                # run its epilogue mid-stream using ONLY DVE + PE.
                if full and it == 7:
                    oT0 = epi_compute("0", 0, P)
                # i 128..239 fully accumulated after tile 14: compute
                # their epilogue off the critical path; store at end.
                if full and it == nit - 2:
                    oTA = epi_compute("A", P, ih - P - ISUB)

            if full:
                # A-store drains right after the last sync-queue mask
                # descriptors; B (the last 16 i's) is the only epilogue
                # on the post-stream critical path.
                nc.sync.dma_start(out=out_d[P:ih - ISUB, :], in_=oTA)
                oTB = epi_compute("B", ih - ISUB, ISUB)
                nc.scalar.dma_start(out=out_d[ih - ISUB:ih, :], in_=oTB)
            else:
                nblk = (ih + P - 1) // P
                for blk in range(nblk):
                    w = min(P, ih - blk * P)
                    oT = epi_compute(f"g{blk}", blk * P, w)
                    deng = nc.sync if blk % 2 == 0 else nc.scalar
                    deng.dma_start(out=out_d[blk * P:blk * P + w, :], in_=oT)
            tpsum_cm.__exit__(None, None, None)
            opsum_cm.__exit__(None, None, None)
            stage1_psum.__exit__(None, None, None)
    nc.finalize()
    return nc


_NC_CACHE = {}


def _get_nc():
    key = "main"
    if key not in _NC_CACHE:
        _NC_CACHE[key] = build_nc()
    return _NC_CACHE[key]


def kernel(s, ef_mask, W, b):
    s = np.ascontiguousarray(s, dtype=np.float32)
    ef_mask = np.ascontiguousarray(ef_mask, dtype=np.float32)
    W = np.ascontiguousarray(W, dtype=np.float32)
    b = np.ascontiguousarray(b, dtype=np.float32)

    nc = _get_nc()
    in_maps = []
    for c in range(N_CORES):
        bb = c // 2
        half = c % 2
        in_maps.append(
            {
                "s": s[bb],
                "w": W,
                "b": b,
                "mask": ef_mask[bb, half * IH:(half + 1) * IH],
            }
        )
    res = run_bass_kernel_spmd(nc, in_maps, list(range(N_CORES))).results
    out = np.empty((B, N, H), dtype=np.float32)
    for c in range(N_CORES):
        bb = c // 2
        half = c % 2
        out[bb, half * IH:(half + 1) * IH] = res[c]["out"]
    return out
